# revision 1
# baseline (speedup 1.0000x reference)
"""MoE combiner kernel for Trainium2 (8 NeuronCores, SPMD).

Computes out[i, d] = sum_e gates[i, e] * expert_outputs[e, d]
  gates:          [16384, 64]  fp32 (top-2 sparse rows, but dense contraction
                                     moves less HBM traffic than a gather)
  expert_outputs: [64, 4096]   fp32
  out:            [16384, 4096] fp32

Sharding: data-parallel over images. Each of the 8 cores computes a
[2048, 4096] slice of the output; the small expert table is replicated.

Math on device: fp32 operands are split host-side into exact fp16
(hi, lo) pairs (hi = fp16(x), lo = fp16(x - hi), after scaling by a power
of two so lo stays in fp16 normal range). The two gate halves are stacked
along the contraction dim (K = 64 experts -> 128 PE rows), so

  psum  = [Ghi; Glo] @ [Ehi; Ehi]   (one K=128 fp16 matmul)
        + [Ghi; Glo] @ [Elo; Elo]   (accumulated, K=128 fp16 matmul)
        = (Ghi + Glo) @ (Ehi + Elo) ~= (G * 2^4) @ (E * 2^8)

and the PSUM->SBUF evacuation rescales by 2^-12. fp16 matmuls stream at
1 column/cycle vs fp32's 4, and the accumulate is fp32 in PSUM, so this
is ~fp32-accurate (~1e-6 rel err) at 4x the PE throughput.
"""

import numpy as np

NUM_EXPERTS = 64
NUM_IMAGES = 16384
D_MODEL = 4096
N_CORES = 8
ROWS = NUM_IMAGES // N_CORES  # 2048 images per core

G_SCALE = 2.0**4   # keeps Glo = fp16(G*16 - fp16(G*16)) in fp16 normal range
E_SCALE = 2.0**8   # same for Elo
OUT_DESCALE = 1.0 / (G_SCALE * E_SCALE)

IMG_TILE = 128          # images per matmul output tile (PSUM partition dim)
N_TILE = 512            # fp32 PSUM bank = 512 floats
OUT_BUFS = 5            # SBUF output staging buffers (bounds DMA in-flight)

_CACHE = {}


def _build_module():
    import concourse.bacc as bacc
    import concourse.mybir as mybir
    import concourse.tile as tile

    # Bacc (not bare Bass): its compile() pipeline runs
    # move_matmul_waits_to_ldweights + generate_event_semaphores, which
    # legalize multi-sem-wait instructions (the ISA allows one sync wait
    # per instruction; walrus rejects more).
    nc = bacc.Bacc("TRN2")
    f16 = mybir.dt.float16
    f32 = mybir.dt.float32

    n_img_tiles = ROWS // IMG_TILE          # 16

    with tile.TileContext(nc) as tc:
        with tc.tile_pool(name="dram", bufs=1, space="DRAM") as dram:
            # One packed input per core, column layout:
            #   [ gatesT hi/lo (ROWS) | Ehi half0 | Elo half0
            #                         | Ehi half1 | Elo half1 ]  (2048 each)
            # so a single leading DMA delivers everything the first half
            # of every image tile needs.
            allin = dram.tile([128, ROWS + 2 * D_MODEL], f16,
                              kind="ExternalInput", name="allin",
                              uniquify=False)
            out = dram.tile([ROWS, D_MODEL], f32, kind="ExternalOutput",
                            name="out", uniquify=False)
            # out[t*128 + p, d] viewed as [p, t, d]: one DMA per image tile
            # covers 128 DRAM rows (16 KiB contiguous each) from one SBUF
            # tile spanning all 128 partitions.
            out_v = out.rearrange("(t p) d -> p t d", p=IMG_TILE)

            with tc.tile_pool(name="const", bufs=1) as cpool, \
                 tc.tile_pool(name="outp", bufs=OUT_BUFS) as outp, \
                 tc.tile_pool(name="psum", bufs=4, space="PSUM") as pspool:
                # Three input DMAs in dependency order: [gt | Ehi0] (what
                # the first matmuls need), then [Elo0], then [Ehi1 | Elo1].
                # Few dma_starts amortize the ~2us fixed per-DMA cost.
                HALF = D_MODEL // 2
                in_sb = cpool.tile([128, ROWS + 2 * D_MODEL], f16,
                                   name="in_sb")
                s1 = ROWS + HALF
                s2 = ROWS + 2 * HALF
                nc.sync.dma_start(out=in_sb[:, :s1], in_=allin[:, :s1])
                nc.sync.dma_start(out=in_sb[:, s1:s2], in_=allin[:, s1:s2])
                nc.sync.dma_start(out=in_sb[:, s2:], in_=allin[:, s2:])
                gt_sb = in_sb[:, :ROWS]
                # eh/el slabs per half: base column of (Ehi, Elo) slab h.
                eh_base = [ROWS, ROWS + 2 * HALF]
                el_base = [ROWS + HALF, ROWS + 3 * HALF]

                # HAM warm-up: ~4us of throwaway matmuls on zeros while the
                # input DMAs are in flight, so the real matmuls start at
                # 2.4 GHz instead of the cold 1.2 GHz gate.
                warm_zero = cpool.tile([128, N_TILE], f16, name="warm_zero")
                nc.vector.memset(warm_zero[:], 0)
                ps_warm = pspool.tile([128, 2 * N_TILE], f32, name="ps")
                for _ in range(10):
                    nc.tensor.matmul(ps_warm[:, :N_TILE],
                                     warm_zero[:, :IMG_TILE], warm_zero[:],
                                     start=True, stop=True)

                PS_W = 2 * N_TILE  # 2 PSUM banks per evacuation copy
                for it in range(n_img_tiles):
                    ot = outp.tile([128, 1, D_MODEL], f32, name="ot")
                    lhsT = gt_sb[:, it * IMG_TILE:(it + 1) * IMG_TILE]
                    for half in range(D_MODEL // PS_W):
                        # Column base of this chunk inside its packed slab.
                        d0 = half * PS_W
                        ehc = eh_base[d0 // HALF] + d0 % HALF
                        elc = el_base[d0 // HALF] + d0 % HALF
                        ps = pspool.tile([128, PS_W], f32, name="ps")
                        # All hi-table matmuls before the lo-table ones so
                        # the first tiles don't stall on the lo load.
                        for q in range(PS_W // N_TILE):
                            ns = slice(ehc + q * N_TILE,
                                       ehc + (q + 1) * N_TILE)
                            qs = slice(q * N_TILE, (q + 1) * N_TILE)
                            nc.tensor.matmul(ps[:, qs], lhsT, in_sb[:, ns],
                                             start=True, stop=False)
                        for q in range(PS_W // N_TILE):
                            ns = slice(elc + q * N_TILE,
                                       elc + (q + 1) * N_TILE)
                            qs = slice(q * N_TILE, (q + 1) * N_TILE)
                            nc.tensor.matmul(ps[:, qs], lhsT, in_sb[:, ns],
                                             start=False, stop=True)
                        # Rescale while evacuating PSUM; split the copy
                        # load between DVE and ACT.
                        dst = ot[:, 0, half * PS_W:(half + 1) * PS_W]
                        if half % 2 == 0:
                            nc.vector.tensor_scalar_mul(dst, ps[:],
                                                        OUT_DESCALE)
                        else:
                            nc.scalar.mul(dst, ps[:], OUT_DESCALE)
                        if it == 0 or it == n_img_tiles - 1:
                            # First tile: per-half stores start the output
                            # stream ~3us earlier. Last tile: a smaller
                            # final DMA shrinks the exposed tail when one
                            # DMA port drains slowly under HBM contention.
                            nc.sync.dma_start(
                                out=out_v[:, it,
                                          half * PS_W:(half + 1) * PS_W],
                                in_=ot[:, 0, half * PS_W:(half + 1) * PS_W])
                    if 0 < it < n_img_tiles - 1:
                        # One 2 MiB DMA per image tile — 1 MiB stores
                        # measured ~12% slower ring throughput.
                        nc.sync.dma_start(out=out_v[:, it:it + 1, :],
                                          in_=ot[:])
    nc.compile()
    return nc


def _get_nc():
    if "nc" not in _CACHE:
        _CACHE["nc"] = _build_module()
    return _CACHE["nc"]


def _split_f16(x):
    hi = x.astype(np.float16)
    lo = (x - hi.astype(np.float32)).astype(np.float16)
    return hi, lo


def _make_in_maps(expert_outputs, gates):
    gs = np.asarray(gates, dtype=np.float32) * np.float32(G_SCALE)
    es = np.asarray(expert_outputs, dtype=np.float32) * np.float32(E_SCALE)
    ghi, glo = _split_f16(gs)
    ehi, elo = _split_f16(es)

    half = D_MODEL // 2
    ehd = np.concatenate([ehi, ehi], axis=0)  # [128, D], rows duplicated
    eld = np.concatenate([elo, elo], axis=0)
    # Packed expert slab: [Ehi h0 | Elo h0 | Ehi h1 | Elo h1]
    eslab = np.concatenate(
        [ehd[:, :half], eld[:, :half], ehd[:, half:], eld[:, half:]], axis=1)

    in_maps = []
    for c in range(N_CORES):
        rs = slice(c * ROWS, (c + 1) * ROWS)
        gt_c = np.concatenate([ghi[rs].T, glo[rs].T], axis=0)  # [128, ROWS]
        allin = np.ascontiguousarray(
            np.concatenate([gt_c, eslab], axis=1))
        in_maps.append({"allin": allin})
    return in_maps


def kernel(expert_outputs: np.ndarray, gates: np.ndarray) -> np.ndarray:
    from concourse.bass_utils import run_bass_kernel_spmd

    nc = _get_nc()
    in_maps = _make_in_maps(expert_outputs, gates)
    res = run_bass_kernel_spmd(nc, in_maps, core_ids=list(range(N_CORES)))
    return np.concatenate([r["out"] for r in res.results], axis=0)



# revision 3
# speedup vs baseline: 1.0528x; 1.0528x over previous
"""MoE combiner kernel for Trainium2 (8 NeuronCores, SPMD).

Computes out[i, d] = sum_e gates[i, e] * expert_outputs[e, d]
  gates:          [16384, 64]  fp32 (top-2 sparse rows, but dense contraction
                                     moves less HBM traffic than a gather)
  expert_outputs: [64, 4096]   fp32
  out:            [16384, 4096] fp32

Sharding: data-parallel over images. Each of the 8 cores computes a
[2048, 4096] slice of the output; the small expert table is replicated.

The kernel is HBM-store-bound: the 32 MiB fp32 output per core dwarfs
the inputs, and the 16 SDMA engines stream it at their aggregate line
rate (~420 GB/s). So inputs are shipped in plain fp16 (the measured
rel err ~3e-4 is far inside the 2e-2 gate; the fp32->fp16 rounding of
gates/experts is the only error source since PSUM accumulates fp32),
which makes the input load a ~1 MiB footnote, and the schedule is
built to keep the store stream saturated from ~3us onward:

  - expert table E [64, 4096] is split-packed as [128, 2048]: columns
    0:2048 on partitions 0-63, columns 2048:4096 on partitions 64-127,
    so its load uses all 16 SBUF AXI ports (a [64, x] load would hit
    only the 8 even ports via the partition->port swizzle).
  - gates^T is duplicated onto both partition halves ([128, 2048]) so
    every (image tile, d chunk) pair has lhsT and rhs on the same
    partition base; matmuls run as K=64 tiles at PE quadrant (0,0) or
    (64,0).
  - the first image tile stores each 512-column chunk as soon as it is
    evacuated (8 small stores), putting first bytes on the store
    stream right after the ~0.4 MiB leading input DMAs; later tiles
    use one 2 MiB store each (small stores measured ~12% slower ring
    throughput); the last tile splits into 4 stores to shrink the
    exposed drain tail.
"""

import numpy as np

NUM_EXPERTS = 64
NUM_IMAGES = 16384
D_MODEL = 4096
N_CORES = 8
ROWS = NUM_IMAGES // N_CORES  # 2048 images per core

IMG_TILE = 128          # images per matmul output tile (PSUM partition dim)
N_TILE = 512            # fp32 PSUM bank = 512 floats
HALF = D_MODEL // 2     # E split-pack boundary
OUT_BUFS = 5            # SBUF output staging buffers (bounds DMA in-flight)

_CACHE = {}


def _build_module():
    import concourse.bacc as bacc
    import concourse.mybir as mybir
    import concourse.tile as tile

    # Bacc (not bare Bass): its compile() pipeline runs
    # move_matmul_waits_to_ldweights + generate_event_semaphores, which
    # legalize multi-sem-wait instructions (the ISA allows one sync wait
    # per instruction; walrus rejects more).
    nc = bacc.Bacc("TRN2")
    f16 = mybir.dt.float16
    f32 = mybir.dt.float32

    n_img_tiles = ROWS // IMG_TILE          # 16
    E0 = ROWS                               # column base of packed E slab

    with tile.TileContext(nc) as tc:
        with tc.tile_pool(name="dram", bufs=1, space="DRAM") as dram:
            # One packed input per core, column layout [128, 2048 + 2048]:
            #   cols 0:2048     gates^T duplicated on both partition halves
            #   cols 2048:4096  E split-packed (d<2048 low / d>=2048 high)
            allin = dram.tile([128, ROWS + HALF], f16,
                              kind="ExternalInput", name="allin",
                              uniquify=False)
            out = dram.tile([ROWS, D_MODEL], f32, kind="ExternalOutput",
                            name="out", uniquify=False)
            # out[t*128 + p, d] viewed as [p, t, d]: one DMA per image tile
            # covers 128 DRAM rows (16 KiB contiguous each) from one SBUF
            # tile spanning all 128 partitions.
            out_v = out.rearrange("(t p) d -> p t d", p=IMG_TILE)

            with tc.tile_pool(name="const", bufs=1) as cpool, \
                 tc.tile_pool(name="outp", bufs=OUT_BUFS) as outp, \
                 tc.tile_pool(name="psum", bufs=8, space="PSUM") as pspool:
                in_sb = cpool.tile([128, ROWS + HALF], f16, name="in_sb")
                # Loads in dependency order: what tile-0 chunk 0 (and, via
                # the high partitions, chunk 4) needs first, then the rest
                # of E, then the remaining gates.
                nc.sync.dma_start(out=in_sb[:, :IMG_TILE],
                                  in_=allin[:, :IMG_TILE])
                nc.sync.dma_start(out=in_sb[:, E0:E0 + N_TILE],
                                  in_=allin[:, E0:E0 + N_TILE])
                nc.sync.dma_start(out=in_sb[:, E0 + N_TILE:],
                                  in_=allin[:, E0 + N_TILE:])
                nc.sync.dma_start(out=in_sb[:, IMG_TILE:E0],
                                  in_=allin[:, IMG_TILE:E0])

                # Chunk order for tile 0: chunks 0 and 4 only need the
                # leading two DMAs, so they go first and the store stream
                # starts while the rest of E is still in flight.
                first_order = [0, 4, 1, 5, 2, 6, 3, 7]
                for it in range(n_img_tiles):
                    ot = outp.tile([128, 1, D_MODEL], f32, name="ot")
                    order = first_order if it == 0 else range(8)
                    for q in order:
                        d0 = q * N_TILE
                        base = 0 if d0 < HALF else 64
                        ecol = E0 + d0 % HALF
                        lhsT = in_sb[base:base + 64,
                                     it * IMG_TILE:(it + 1) * IMG_TILE]
                        rhs = in_sb[base:base + 64, ecol:ecol + N_TILE]
                        ps = pspool.tile([128, N_TILE], f32, name="ps")
                        nc.tensor.matmul(ps[:], lhsT, rhs,
                                         start=True, stop=True)
                        # Evacuate PSUM; split the copy load between DVE
                        # and ACT so neither becomes the per-tile limiter.
                        dst = ot[:, 0, d0:d0 + N_TILE]
                        if q % 2 == 0:
                            nc.vector.tensor_scalar_mul(dst, ps[:], 1.0)
                        else:
                            nc.scalar.mul(dst, ps[:], 1.0)
                        if it == 0:
                            nc.sync.dma_start(
                                out=out_v[:, it, d0:d0 + N_TILE], in_=dst)
                    if it == n_img_tiles - 1:
                        # Smaller final DMAs shrink the exposed tail when
                        # one DMA port drains slowly under HBM contention.
                        for h in range(4):
                            cs = slice(h * D_MODEL // 4,
                                       (h + 1) * D_MODEL // 4)
                            nc.sync.dma_start(out=out_v[:, it, cs],
                                              in_=ot[:, 0, cs])
                    elif it > 0:
                        # One 2 MiB DMA per image tile — 1 MiB stores
                        # measured ~12% slower ring throughput.
                        nc.sync.dma_start(out=out_v[:, it:it + 1, :],
                                          in_=ot[:])
    nc.compile()
    return nc


def _get_nc():
    if "nc" not in _CACHE:
        _CACHE["nc"] = _build_module()
    return _CACHE["nc"]


def _make_in_maps(expert_outputs, gates):
    g16 = np.asarray(gates, dtype=np.float16)
    e16 = np.asarray(expert_outputs, dtype=np.float16)
    # E split-pack: [E[:, :2048] ; E[:, 2048:]] -> [128, 2048]
    eslab = np.concatenate([e16[:, :HALF], e16[:, HALF:]], axis=0)

    in_maps = []
    for c in range(N_CORES):
        gt = g16[c * ROWS:(c + 1) * ROWS].T          # [64, 2048]
        allin = np.ascontiguousarray(
            np.concatenate([np.concatenate([gt, gt], axis=0), eslab],
                           axis=1))
        in_maps.append({"allin": allin})
    return in_maps


def kernel(expert_outputs: np.ndarray, gates: np.ndarray) -> np.ndarray:
    from concourse.bass_utils import run_bass_kernel_spmd

    nc = _get_nc()
    in_maps = _make_in_maps(expert_outputs, gates)
    res = run_bass_kernel_spmd(nc, in_maps, core_ids=list(range(N_CORES)))
    return np.concatenate([r["out"] for r in res.results], axis=0)
